# revision 2
# baseline (speedup 1.0000x reference)
"""Multi-head causal self-attention (B=4, S=2048, D=512, H=8) on 8 Trainium2
NeuronCores — compensated-fp8 version.

Sharding: core c = batch c//2, head-group c%2 (4 heads / 256 dout slice).
Disjoint outputs, no collectives.

Key ideas vs the bf16 baseline:
  - All projections run as fp8e4m3 DoubleRow (DR) matmuls with hi/lo error
    compensation: x = hi + lo (both fp8, host-split, power-of-2 pre-scaled so
    the lo residual stays clear of the fp8 denormal floor). Terms
    xhi@whi + xlo@whi + xhi@wlo accumulate in one PSUM group: 6 DR steps
    replace 4 bf16 steps at half the per-column cost (3 vs 4 cycles/col).
  - Scores compute the exact product (Khi+Klo).(Qhi+Qlo) with TWO DR matmuls
    over j-interleaved [p, {hi,lo}, s] projections: a straight pass
    (Khi.Qhi + Klo.Qlo) plus a j-reversed pass (Khi.Qlo + Klo.Qhi), both
    accumulating into the same PSUM group. Same cost as bf16 scores but in
    fp8 with ~bf16 accuracy, and no extra operand staging.
  - PV runs transposed: acc[q,dv] += et[k,q]^T @ vaug[k,dv] — out rows = 128
    q's, only 65 streamed columns per (kt,qt) tile, halving PV tensor time
    and leaving the output in [q, dv] layout (no host transpose).
  - PSUM accumulation groups are strictly sequential (matmul start=True
    clears has_written bank-wide, corrupting any other open group in the
    bank): PV accumulates one q-tile at a time over its full k range,
    rotating 4 accumulator slots; et chunks stay resident (epool bufs=14).
  - exp on ACT in 1024-col chunks (2 PSUM banks, double buffered), bf16 out;
    softmax denominator rides as vaug's 65th (ones) column.
  - Causal diag-block masking runs on GPSIMD; hi/lo splits, V-bias and
    output copies on DVE — ACT stays exp-only (the critical engine).
  - DMA count is kept low (HWDGE descriptor generation is a single shared
    ~630ns/transfer device): inputs load in halves, outputs stage into
    [128, 8, 65] tiles DMA'd once per (head, half).
  - Linear-bias handling: bk drops (softmax shift invariance), bv is added
    on-device into vaug, bq enters logits via c_k = bq.(k+bk)/8 computed by
    tiny DR matvecs feeding exp's per-partition bias (only exercised when
    bq != 0, in which case exp chunks are restricted to single k-tiles).
"""

import numpy as np
import ml_dtypes

from concourse import bacc, mybir
from concourse.instruction_name_ordered_set import InstructionNameOrderedSet
from concourse.tile import TileContext
from concourse.bass_utils import run_bass_kernel_spmd

BF16 = mybir.dt.bfloat16
FP8 = mybir.dt.float8e4
F32 = mybir.dt.float32
AF = mybir.ActivationFunctionType
ALU = mybir.AluOpType
DRM = mybir.MatmulPerfMode.DoubleRow
E4 = ml_dtypes.float8_e4m3
BFNP = ml_dtypes.bfloat16

B, S, D = 4, 2048, 512
H, HD, HPC, DSL = 8, 64, 4, 256
N_CORES = 8
NT = S // 128  # 16 q/k tiles per sequence

SX, SW, SQ = 8.0, 256.0, 8.0
ASCALE = SQ / (SX * SW)          # psum_qk -> scaled q/k (2^-8)
VSCALE = 1.0 / (SX * SW)         # psum_v -> natural v (2^-11)
ESCALE = 1.0 / (8.0 * SQ * SQ)   # exp scale on scores (1/512)
CHUNK = 1024                     # exp chunk width (2 PSUM banks)

PASSES = 1  # kept for harness compat; only 1 supported


def plan_chunks(qh, per_kt=False, qrev=False):
    """Units are 128-col (kt, qt) tiles, kt-major; packed into <=8-unit
    chunks. per_kt=True additionally breaks chunks at kt boundaries (needed
    when exp carries a per-k bias). qrev=True orders units qt-descending
    (kt-ascending within a qt) so late PV groups complete as early as
    possible — used for the final half to shorten the drain tail."""
    qt0 = 8 * qh
    units = []
    if qrev == "asc":  # q-ascending column order: minimal input prefix,
        # and one PV group completes per column end (no group bunching)
        for qt in range(qt0, qt0 + 8):
            for kt in range(0, qt + 1):
                units.append((kt, qt))
    elif qrev:
        for qt in range(qt0 + 7, qt0 - 1, -1):
            for kt in range(0, qt + 1):
                units.append((kt, qt))
    else:
        for kt in range(8 + 8 * qh):
            for qt in range(max(qt0, kt), qt0 + 8):
                units.append((kt, qt))
    chunks, cur = [], []
    for u in units:
        if cur and (len(cur) == 8 or (per_kt and u[0] != cur[-1][0])):  # noqa
            chunks.append(cur)
            cur = []
        cur.append(u)
    chunks.append(cur)
    return chunks


def build_nc(bias_mode=False, debug=False):
    nc = bacc.Bacc("TRN2", target_bir_lowering=False)

    xs, ws = {}, {}
    for t in "qkv":
        for p in ("hi", "lo"):
            xs[t, p] = nc.declare_dram_parameter(f"x{t}{p}", [D, S], FP8, isOutput=False)
            ws[t, p] = nc.declare_dram_parameter(f"w{t}{p}", [D, DSL], FP8, isOutput=False)
    smallp = nc.declare_dram_parameter("smallp", [128, 386], F32, isOutput=False)
    mbqp = nc.declare_dram_parameter("mbqp", [128, 16], FP8, isOutput=False)
    out_t = nc.declare_dram_parameter("out_t", [HPC * S, HD + 1], F32, isOutput=True)
    if debug:
        dbg_pq = nc.declare_dram_parameter("dbg_pq", [128, 2 * S], FP8, isOutput=True)
        dbg_pk = nc.declare_dram_parameter("dbg_pk", [128, 2 * S], FP8, isOutput=True)
        dbg_va = nc.declare_dram_parameter("dbg_va", [128, NT * HPC * (HD + 1)], BF16, isOutput=True)
        dbg_et = nc.declare_dram_parameter("dbg_et", [128, 5 * CHUNK], BF16, isOutput=True)

    with TileContext(nc) as tc:
        with tc.tile_pool(name="const", bufs=1) as cpool:
            x_sb = {k: cpool.tile([128, 4, S], FP8, tag=f"x{k[0]}{k[1]}", name=f"x{k[0]}{k[1]}") for k in xs}
            w_sb = {k: cpool.tile([128, 4, DSL], FP8, tag=f"w{k[0]}{k[1]}", name=f"w{k[0]}{k[1]}") for k in ws}
            small = cpool.tile([128, 386], F32, tag="small")
            bvb = small[:, 0:256]
            mask = small[:, 256:320].bitcast(BF16)      # [128, 128] triu
            bqbk = small[:, 322:386]                    # [128, 64]: 4 heads x 16
            # mbq_t[p, h, j, v]: v=0 straight (hi,lo), v=1 swapped (lo,hi);
            # per-head content sits at partitions [64*(h%2), 64*(h%2)+64)
            mbq_t = cpool.tile([128, 4, 2, 2], FP8, tag="mbq", name="mbq_t")
            # j-interleaved projections: [p, {hi,lo}, s], p = dout row in mc
            PQ = [cpool.tile([128, 2, S], FP8, tag=f"PQ{m}", name=f"PQ{m}") for m in range(2)]
            PK = [cpool.tile([128, 2, S], FP8, tag=f"PK{m}", name=f"PK{m}") for m in range(2)]
            vaug = cpool.tile([128, NT, HPC, HD + 1], BF16, tag="vaug")
            cb = cpool.tile([128, 64], F32, tag="cb")   # 4 heads x 16 exp biases
            warm = cpool.tile([128, 512], BF16, tag="warm")

            nc.vector.memset(vaug[:, :, :, HD : HD + 1], 1.0)
            nc.vector.memset(warm[:], 0.0)

            def load_w(t):
                for p in ("hi", "lo"):
                    nc.sync.dma_start(
                        w_sb[t, p][:],
                        ws[t, p][:].rearrange("(c p) m -> p c m", p=128),
                    )

            def load_x(t, s0, s1):
                for p in ("hi", "lo"):
                    nc.sync.dma_start(
                        x_sb[t, p][:, :, s0:s1],
                        xs[t, p][:, s0:s1].rearrange("(c p) s -> p c s", p=128),
                    )

            load_w("k")
            load_x("k", 0, 512)
            load_w("q")
            load_x("q", 0, 512)
            load_x("k", 512, 1024)
            load_x("q", 512, 1024)
            nc.sync.dma_start(small[:], smallp[:])
            nc.sync.dma_start(
                mbq_t[:], mbqp[:].rearrange("p (h j v) -> p h j v", j=2, v=2))
            load_w("v")
            load_x("v", 0, 1024)
            load_x("k", 1024, 2048)
            load_x("q", 1024, 2048)
            load_x("v", 1024, 2048)

            with (
                tc.tile_pool(name="projp", bufs=2, space="PSUM") as projp,
                tc.tile_pool(name="spool", bufs=2, space="PSUM") as spool,
                tc.tile_pool(name="apool", bufs=1, space="PSUM") as apool,
                tc.tile_pool(name="cxp", bufs=1, space="PSUM") as cxp,
                tc.tile_pool(name="epool", bufs=14) as epool,
                tc.tile_pool(name="otp", bufs=3) as otp,
            ):
                cx = cxp.tile([128, 16], F32, tag="cx")

                # Chain ALL PE matmuls in emission order via no-sync deps.
                # The PE executes in order anyway, so this costs nothing at
                # runtime; it pins Tile's PE stream to the emission order so
                # the PSUM zero-region analysis (start=True poisons a whole
                # bank) holds deterministically.
                _pe_prev = [None]

                def mm(*args, **kw):
                    m = nc.tensor.matmul(*args, **kw)
                    if _pe_prev[0] is not None:
                        dep = InstructionNameOrderedSet()
                        dep.add(_pe_prev[0].ins.name)
                        m.ins.add_nosync_dependencies_from(dep)
                    _pe_prev[0] = m
                    return m

                # PE p-state warmup: keep the array busy during input DMAs
                for i in range(10):
                    wp = projp.tile([128, 512], F32, tag="pqk", name="warm")
                    mm(wp[:], warm[:, 0:128], warm[:], start=True, stop=True)

                def proj_qk(t, mc, sc):
                    ps = projp.tile([128, 512], F32, tag="pqk", name="psqk")
                    xh, xl = x_sb[t, "hi"], x_sb[t, "lo"]
                    wh, wl = w_sb[t, "hi"], w_sb[t, "lo"]
                    steps = [(xh, wh, 0), (xh, wh, 1), (xl, wh, 0),
                             (xl, wh, 1), (xh, wl, 0), (xh, wl, 1)]
                    m0, s0 = 128 * mc, 512 * sc
                    for i, (xt, wt, a) in enumerate(steps):
                        mm(
                            ps[:],
                            wt[:, 2 * a : 2 * a + 2, m0 : m0 + 128],
                            xt[:, 2 * a : 2 * a + 2, s0 : s0 + 512],
                            start=(i == 0),
                            stop=(i == 5),
                            perf_mode=DRM,
                        )
                    P = PQ[mc] if t == "q" else PK[mc]
                    nc.vector.tensor_scalar_mul(P[:, 0, s0 : s0 + 512], ps[:], ASCALE)
                    nc.vector.scalar_tensor_tensor(
                        P[:, 1, s0 : s0 + 512], ps[:], ASCALE,
                        P[:, 0, s0 : s0 + 512], ALU.mult, ALU.subtract,
                    )

                def proj_v(st):
                    ps = projp.tile([128, 8, 64], F32, tag="pqk", name="psv")
                    xh, xl = x_sb["v", "hi"], x_sb["v", "lo"]
                    wh, wl = w_sb["v", "hi"], w_sb["v", "lo"]
                    steps = [(xh, wh, 0), (xh, wh, 1), (xl, wh, 0),
                             (xl, wh, 1), (xh, wl, 0), (xh, wl, 1)]
                    s0 = 128 * st
                    for i, (xt, wt, a) in enumerate(steps):
                        mm(
                            ps[:, 0:4, :],
                            xt[:, 2 * a : 2 * a + 2, s0 : s0 + 128],
                            wt[:, 2 * a : 2 * a + 2, :],
                            start=(i == 0),
                            stop=(i == 5),
                            perf_mode=DRM,
                        )
                    nc.vector.scalar_tensor_tensor(
                        vaug[:, st, :, 0:HD], ps[:, 0:4, :], VSCALE,
                        bvb, ALU.mult, ALU.add,
                    )

                def c_matvec(h, kts):
                    mc, P0 = h // 2, 64 * (h % 2)
                    for kt in kts:
                        for v in range(2):
                            mm(
                                cx[:, kt : kt + 1],
                                PK[mc][P0 : P0 + 64, :, 128 * kt : 128 * kt + 128],
                                mbq_t[P0 : P0 + 64, h, :, v : v + 1],
                                start=(v == 0), stop=(v == 1),
                                perf_mode=DRM, skip_group_check=True,
                            )

                def c_bias(h, qh):
                    o = 16 * h + 8 * qh
                    nc.vector.scalar_tensor_tensor(
                        cb[:, o : o + 8], cx[:, 8 * qh : 8 * qh + 8],
                        ESCALE, bqbk[:, o : o + 8], ALU.mult, ALU.add,
                    )

                def plan_for(h, qh):
                    if qh == 1:
                        qr = True if h == 3 else "asc"
                    else:
                        qr = "asc"
                    return plan_chunks(qh, per_kt=bias_mode, qrev=qr)

                pending_sl = {}

                def scores_for(h, qh, ch):
                    mc, P0 = h // 2, 64 * (h % 2)
                    sl = spool.tile([128, CHUNK], F32, tag="sl", name="sl")
                    runs, i = [], 0
                    while i < len(ch):
                        j = i
                        while j + 1 < len(ch) and ch[j + 1][0] == ch[i][0]:
                            j += 1
                        runs.append((ch[i][0], ch[i][1], 128 * i, 128 * (j - i + 1)))
                        i = j + 1
                    for kt, qta, off, w in runs:
                        q0 = 128 * qta
                        c0 = 0
                        while c0 < w:  # split at PSUM bank boundaries
                            c1 = min(w, (off + c0) // 512 * 512 + 512 - off)
                            lhsT = PK[mc][P0 : P0 + 64, :, 128 * kt : 128 * kt + 128]
                            mm(
                                sl[:, off + c0 : off + c1],
                                lhsT,
                                PQ[mc][P0 : P0 + 64, :, q0 + c0 : q0 + c1],
                                start=True, stop=False,
                                perf_mode=DRM, skip_group_check=True,
                            )
                            mm(
                                sl[:, off + c0 : off + c1],
                                lhsT,
                                PQ[mc][P0 : P0 + 64, ::-1, q0 + c0 : q0 + c1],
                                start=False, stop=True,
                                perf_mode=DRM, skip_group_check=True,
                            )
                            c0 = c1
                    return sl

                def attention(h, qh, defer, tail=(), post=None):
                    post = post or {}
                    chunks = plan_for(h, qh)
                    # rotating accumulator slots: exactly ONE accumulation
                    # group is open at any time (start=True clears
                    # has_written bank-wide)
                    acc = apool.tile([128, 4, HD + 1], F32, tag="acc", name="acc")
                    ot = otp.tile([128, 8, HD + 1], F32, tag="ot", name="ot")
                    qt0 = 8 * qh
                    pos = {qt: [] for qt in range(qt0, qt0 + 8)}
                    for ci_, ch_ in enumerate(chunks):
                        for ui_, (kt_, qt_) in enumerate(ch_):
                            pos[qt_].append((ci_, ui_, kt_))
                    done_at = {qt_: p[-1][0] for qt_, p in pos.items()}
                    ets = {}
                    done_qts, dma_done = set(), set()

                    sl = pending_sl.pop((h, qh), None)
                    if sl is None:
                        sl = scores_for(h, qh, chunks[0])
                    for ci, ch in enumerate(chunks):
                        We = 128 * len(ch)
                        et = epool.tile([128, CHUNK], BF16, tag="et", name="et")
                        ets[ci] = et
                        bias = cb[:, 16 * h + ch[0][0] : 16 * h + ch[0][0] + 1] if bias_mode else 0.0
                        nc.scalar.activation(et[:, 0:We], sl[:, 0:We], AF.Exp,
                                             bias=bias, scale=ESCALE)
                        for ui, (kt, qt) in enumerate(ch):
                            if kt == qt:
                                u = slice(128 * ui, 128 * ui + 128)
                                nc.gpsimd.tensor_mul(et[:, u], et[:, u], mask)
                        if debug and h == 0 and qh == 0:
                            nc.sync.dma_start(
                                dbg_et[:, CHUNK * ci : CHUNK * ci + We], et[:, 0:We])
                        for work in defer.get((qh, ci), ()):
                            work()
                        if ci + 1 < len(chunks):
                            sl = scores_for(h, qh, chunks[ci + 1])
                        if ci == len(chunks) - 2 or len(chunks) == 1:
                            for w_ in tail:  # stitch next half's front work
                                w_()
                        for work in post.get((qh, ci), ()):
                            work()
                        if ci == len(chunks) - 1:
                            ready = [q for q in pos if done_at[q] >= ci - 1]
                        else:
                            ready = [q for q in pos if done_at[q] == ci - 1]
                        ready.sort(key=lambda q: done_at[q])
                        for qt in ready:
                            lq = qt - qt0
                            oap = acc[:, lq % 4, :]
                            n = len(pos[qt])
                            for j, (cj, uj, kt) in enumerate(pos[qt]):
                                mm(
                                    oap,
                                    ets[cj][:, 128 * uj : 128 * uj + 128],
                                    vaug[:, kt, h, :],
                                    start=(j == 0), stop=(j == n - 1),
                                    skip_group_check=True,
                                )
                            nc.vector.tensor_copy(ot[:, lq, :], oap)
                            done_qts.add(lq)
                            r0 = S * h + 1024 * qh
                            for half_ in (0, 1):
                                grp_ = {0, 1, 2, 3} if half_ == 0 else {4, 5, 6, 7}
                                if grp_ <= done_qts and half_ not in dma_done:
                                    dma_done.add(half_)
                                    nc.sync.dma_start(
                                        out_t[r0 + 512 * half_ : r0 + 512 * half_ + 512, :]
                                        .rearrange("(qt p) d -> p qt d", p=128),
                                        ot[:, 4 * half_ : 4 * half_ + 4, :],
                                    )

                # ---- prologue ----
                proj_qk("k", 0, 0)
                proj_qk("q", 0, 0)
                c_matvec(0, range(4))

                def d(fn, *a):
                    return lambda: fn(*a)

                def mv_bias00():
                    c_matvec(0, range(4, 8))
                    if bias_mode:
                        c_bias(0, 0)

                halves = [(0, 0), (1, 0), (0, 1), (2, 0),
                          (1, 1), (3, 0), (2, 1), (3, 1)]
                defers = {
                    # pre-defers run before the next chunk's scores: only
                    # work that later scores/exp depend on goes here
                    (0, 0): {
                        (0, 0): [d(proj_qk, "k", 0, 1), d(proj_qk, "q", 0, 1)],
                        (0, 1): [mv_bias00],
                    },
                    (1, 0): {
                        (0, 0): [d(proj_qk, "k", 0, 2)],
                        (0, 1): [d(proj_qk, "q", 0, 2)],
                        (0, 2): [d(proj_qk, "k", 0, 3)],
                        (0, 3): [d(proj_qk, "q", 0, 3)],
                    },
                    (0, 1): {
                        (1, 0): [d(proj_qk, "k", 1, 0)],
                        (1, 1): [d(proj_qk, "q", 1, 0)],
                        (1, 2): [d(proj_qk, "k", 1, 1)],
                        (1, 3): [d(proj_qk, "q", 1, 1)],
                    },
                    (2, 0): {
                        (0, 0): [d(proj_qk, "k", 1, 2)],
                        (0, 1): [d(proj_qk, "q", 1, 2)],
                        (0, 2): [d(proj_qk, "k", 1, 3)],
                        (0, 3): [d(proj_qk, "q", 1, 3)],
                    },
                }
                posts = {
                    # post-defers run after the next chunk's scores: V-proj
                    # feeds only PV groups, never scores
                    (0, 0): {
                        (0, 0): [d(proj_v, 0), d(proj_v, 1), d(proj_v, 2)],
                        (0, 1): [d(proj_v, 3), d(proj_v, 4)],
                        (0, 2): [d(proj_v, 5)],
                        (0, 3): [d(proj_v, 6), d(proj_v, 7)],
                    },
                    (0, 1): {
                        (1, 0): [d(proj_v, 8)],
                        (1, 1): [d(proj_v, 9)],
                        (1, 2): [d(proj_v, 10)],
                        (1, 3): [d(proj_v, 11)],
                        (1, 4): [d(proj_v, 12)],
                        (1, 5): [d(proj_v, 13)],
                        (1, 6): [d(proj_v, 14)],
                        (1, 7): [d(proj_v, 15)],
                    },
                }

                def stitch(nh, nqh):
                    # emitted right after the current half's final exp:
                    # prepare the NEXT half's biases and first scores so ACT
                    # never waits at the half boundary
                    def f():
                        c_matvec(nh, range(8) if nqh == 0 else range(8, 16))
                        if bias_mode:
                            c_bias(nh, nqh)
                        pending_sl[(nh, nqh)] = scores_for(
                            nh, nqh, plan_for(nh, nqh)[0])
                    return f

                for i, (h, qh) in enumerate(halves):
                    tail = ()
                    if i + 1 < len(halves):
                        tail = (stitch(*halves[i + 1]),)
                    attention(h, qh, defers.get((h, qh), {}), tail=tail,
                              post=posts.get((h, qh), {}))
                    if debug and (h, qh) == (0, 1):
                        nc.sync.dma_start(dbg_pq[:], PQ[0][:].rearrange("p j s -> p (j s)"))
                        nc.sync.dma_start(dbg_pk[:], PK[0][:].rearrange("p j s -> p (j s)"))
                        nc.sync.dma_start(dbg_va[:], vaug[:].rearrange("p a b c -> p (a b c)"))

    nc.finalize()
    return nc


_NC_CACHE = {}


def _get_nc(bias_mode=False):
    if bias_mode not in _NC_CACHE:
        _NC_CACHE[bias_mode] = build_nc(bias_mode)
    return _NC_CACHE[bias_mode]


def _split8(x, s):
    xs = np.asarray(x, np.float32) * s
    hi = xs.astype(E4)
    lo = (xs - hi.astype(np.float32)).astype(E4)
    return hi, lo


def make_in_maps(query, key, value, Wq, bq, Wk, bk, Wv, bv):
    query, key, value = (np.asarray(v, np.float32) for v in (query, key, value))
    Wq, Wk, Wv = (np.asarray(v, np.float32) for v in (Wq, Wk, Wv))
    bq, bk, bv = (np.asarray(v, np.float32) for v in (bq, bk, bv))
    mask = np.triu(np.ones((128, 128), np.float32)).astype(BFNP)

    in_maps = []
    for c in range(N_CORES):
        b, g = c // 2, c % 2
        sl = slice(DSL * g, DSL * g + DSL)
        m = {}
        for t, xfull, wfull in (("q", query, Wq), ("k", key, Wk), ("v", value, Wv)):
            xh, xl = _split8(np.ascontiguousarray(xfull[b].T), SX)
            wh, wl = _split8(np.ascontiguousarray(wfull[sl].T), SW)
            m[f"x{t}hi"], m[f"x{t}lo"] = xh, xl
            m[f"w{t}hi"], m[f"w{t}lo"] = wh, wl

        small = np.zeros((128, 386), np.float32)
        small[:, 0:256] = np.tile(bv[sl][None, :], (128, 1))
        small[:, 256:320] = np.ascontiguousarray(mask).view(np.float32)
        mbq8 = np.zeros((128, 4, 2, 2), E4)
        bqbk = np.zeros((128, 64), np.float32)
        for hl in range(HPC):
            bqh = bq[sl][64 * hl : 64 * hl + 64]
            bkh = bk[sl][64 * hl : 64 * hl + 64]
            hi, lo = _split8(bqh, SQ)
            P0 = 64 * (hl % 2)
            r = slice(P0, P0 + 64)
            mbq8[r, hl, 0, 0], mbq8[r, hl, 1, 0] = hi, lo   # straight
            mbq8[r, hl, 0, 1], mbq8[r, hl, 1, 1] = lo, hi   # swapped
            bqbk[:, 16 * hl : 16 * hl + 16] = float(np.dot(bqh, bkh)) / 8.0
        small[:, 322:386] = bqbk
        m["smallp"] = small
        m["mbqp"] = np.ascontiguousarray(mbq8.reshape(128, 16))
        in_maps.append(m)
    return in_maps


def assemble_output(results):
    out = np.empty((B, S, D), np.float32)
    for c in range(N_CORES):
        b, g = c // 2, c % 2
        ot = results[c]["out_t"].reshape(HPC, S, HD + 1)
        for hl in range(HPC):
            h = HPC * g + hl
            blk = ot[hl]
            out[b, :, HD * h : HD * h + HD] = blk[:, 0:HD] / blk[:, HD : HD + 1]
    return out


def run(trace=False, **inputs):
    bias_mode = bool(np.any(np.asarray(inputs["bq"])))
    nc = _get_nc(bias_mode)
    in_maps = make_in_maps(**inputs)
    res = run_bass_kernel_spmd(nc, in_maps, list(range(N_CORES)), trace=trace)
    return assemble_output(res.results), res


def kernel(**inputs) -> np.ndarray:
    out, _ = run(trace=False, **inputs)
    return out


# revision 3
# speedup vs baseline: 1.0571x; 1.0571x over previous
"""Multi-head causal self-attention (B=4, S=2048, D=512, H=8) on 8 Trainium2
NeuronCores.

Sharding: core c handles batch b = c//2 and a 4-head group g = c%2
(heads 4g..4g+3, i.e. output-feature slice [256g, 256g+256)).  Each core's
output is a disjoint slice of the full output, so no collectives are needed.

Device kernel layout choices (per core):
  - inputs are passed transposed+bf16 (xT = x.T : [din, S]) so the
    projection matmuls can contract din on the partition dim.
  - Q,K are produced transposed ("QT/KT" = [dout, S]); attention scores are
    computed transposed: ST[k, q] = sum_d KT[d,k] * QT[d,q], which makes the
    softmax denominator and PV matmul contract over k on partitions.
  - softmax skips the max-subtraction: logits = s/8 with |s/8| <~ 6 for this
    problem's N(0,1)-ish inputs, safely inside exp's fp32 range.  exp runs on
    the scalar engine straight out of PSUM.
  - V is augmented with a ones-column, so the PV matmul accumulates both
    out^T[dv, q] and the softmax denominator (row 64) in one pass.
  - normalization (divide by denom) + final transpose happen on the host
    during the gather step.
"""

import numpy as np
import ml_dtypes

from concourse import bacc, mybir
from concourse.tile import TileContext
from concourse.bass_utils import run_bass_kernel_spmd

BF16 = mybir.dt.bfloat16
F32 = mybir.dt.float32
AF = mybir.ActivationFunctionType
BFNP = ml_dtypes.bfloat16

B, S, D = 4, 2048, 512
H, HD = 8, 64
HPC = 4                   # heads per core
DSL = HPC * HD            # 256-wide output-feature slice per core
N_CORES = 8
SCALE = float(HD) ** 0.5  # 8.0
QH_W = 1024               # q processed in two halves of 1024


# timing instrumentation only: emit the compute body N times (identical
# output; wall-clock delta between variants isolates device compute time)
PASSES = 1


def build_nc():
    nc = bacc.Bacc("TRN2", target_bir_lowering=False)

    qT = nc.declare_dram_parameter("qT", [D, S], BF16, isOutput=False)
    kTd = nc.declare_dram_parameter("kTd", [D, S], BF16, isOutput=False)
    vT = nc.declare_dram_parameter("vT", [D, S], BF16, isOutput=False)
    wqT = nc.declare_dram_parameter("wqT", [D, DSL], BF16, isOutput=False)
    wkT = nc.declare_dram_parameter("wkT", [D, DSL], BF16, isOutput=False)
    wvT = nc.declare_dram_parameter("wvT", [D, DSL], BF16, isOutput=False)
    # packed small tensors: [0:2]=bq, [2:4]=bk, [4:260]=bvb, [260:324]=mask(bf16 bits)
    smallp = nc.declare_dram_parameter("smallp", [128, 324], F32, isOutput=False)
    # rows [65h, 65h+64) = unnormalized out^T for head h; row 65h+64 = denom
    out_t = nc.declare_dram_parameter(
        "out_t", [HPC * (HD + 1), S], F32, isOutput=True
    )

    with TileContext(nc) as tc:
        with tc.tile_pool(name="const", bufs=1) as cpool:
            qT_sb = cpool.tile([128, 4, S], BF16, tag="qT_sb")
            kT_sb = cpool.tile([128, 4, S], BF16, tag="kT_sb")
            vT_sb = cpool.tile([128, 4, S], BF16, tag="vT_sb")
            wq_sb = cpool.tile([128, 4, DSL], BF16, tag="wq_sb")
            wk_sb = cpool.tile([128, 4, DSL], BF16, tag="wk_sb")
            wv_sb = cpool.tile([128, 4, DSL], BF16, tag="wv_sb")
            small_sb = cpool.tile([128, 324], F32, tag="small_sb")
            bq_sb = small_sb[:, 0:2]
            bk_sb = small_sb[:, 2:4]
            bvb_sb = small_sb[:, 4:260]
            mask_sb = small_sb[:, 260:324].bitcast(BF16)
            # projected tensors: chunk dim = head pair (dout 128-chunk)
            QT_sb = cpool.tile([128, 2, S], BF16, tag="QT_sb")
            KT_sb = cpool.tile([128, 2, S], BF16, tag="KT_sb")
            # V with ones column: [k-part, head, k-tile, dv+1]
            vaug_sb = cpool.tile([128, HPC, 16, HD + 1], BF16, tag="vaug_sb")

            # only the ones-column needs init; cols 0..63 are written by proj_v
            nc.vector.memset(vaug_sb[:, :, :, HD : HD + 1], 1.0)

            def load_w(w_sb, wsrc, eng):
                eng.dma_start(w_sb[:], wsrc[:].rearrange("(c p) m -> p c m", p=128))

            def load_x(dstt, srcd, sq, eng):
                s0 = 512 * sq
                eng.dma_start(
                    dstt[:, :, s0 : s0 + 512],
                    srcd[:, s0 : s0 + 512].rearrange("(c p) s -> p c s", p=128),
                )

            # loads ordered by when the first attention tiles need them
            _Q, _K, _V = (qT_sb, qT), (kT_sb, kTd), (vT_sb, vT)
            load_w(wv_sb, wvT, nc.sync)
            load_x(*_V, 0, nc.sync)
            load_w(wk_sb, wkT, nc.sync)
            load_x(*_K, 0, nc.sync)
            nc.sync.dma_start(small_sb[:], smallp[:])
            load_w(wq_sb, wqT, nc.sync)
            load_x(*_Q, 0, nc.sync)
            load_x(*_Q, 1, nc.sync)
            for xt, sq in ((_K, 1), (_V, 1), (_Q, 2), (_Q, 3),
                           (_K, 2), (_K, 3), (_V, 2), (_V, 3)):
                load_x(*xt, sq, nc.sync)

            # ---- projections + attention, interleaved ----
            # PSUM budget: ppool 2x1 + spool 2x2 + apool 1x2 = 8 banks
            with (
                tc.tile_pool(name="ppsum", bufs=2, space="PSUM") as ppool,
                tc.tile_pool(name="spsum", bufs=2, space="PSUM") as spool,
                tc.tile_pool(name="apsum", bufs=1, space="PSUM") as apool,
                tc.tile_pool(name="epool", bufs=7) as epool,
                tc.tile_pool(name="opool", bufs=3) as opool,
            ):

                def proj_v_st(st):
                    ps = ppool.tile([128, 512], F32, tag="pproj", name="psv")
                    for dc in range(4):
                        nc.tensor.matmul(
                            ps[:, 0:DSL],
                            vT_sb[:, dc, 128 * st : 128 * st + 128],
                            wv_sb[:, dc, :],
                            start=(dc == 0),
                            stop=(dc == 3),
                        )
                    for hh in range(HPC):
                        nc.vector.tensor_add(
                            vaug_sb[:, hh, st, 0:HD],
                            ps[:, HD * hh : HD * hh + HD],
                            bvb_sb[:, HD * hh : HD * hh + HD],
                        )

                QSRC = (wq_sb, bq_sb, qT_sb, QT_sb)
                KSRC = (wk_sb, bk_sb, kT_sb, KT_sb)

                def proj_qk_tile(mc, sc, src):
                    w_sb, b_sb, x_sb, dst = src
                    ps = ppool.tile([128, 512], F32, tag="pproj", name="psqk")
                    for dc in range(4):
                        nc.tensor.matmul(
                            ps[:],
                            w_sb[:, dc, 128 * mc : 128 * mc + 128],
                            x_sb[:, dc, 512 * sc : 512 * sc + 512],
                            start=(dc == 0),
                            stop=(dc == 3),
                        )
                    nc.vector.tensor_scalar_add(
                        dst[:, mc, 512 * sc : 512 * sc + 512],
                        ps[:],
                        b_sb[:, mc : mc + 1],
                    )

                def attn_head(h, sched=None):
                    sched = sched or {}
                    mc, prow = h // 2, 64 * (h % 2)
                    GROUPS = {
                        0: [(0,), (1,), (2,), (3,), (4, 5), (6, 7)],
                        1: [(k,) for k in range(12)] + [(12, 13), (14, 15)],
                    }

                    def geom(qh, kt):
                        Q0 = QH_W * qh
                        K0 = 128 * kt
                        qlo = max(Q0, K0)
                        return K0, qlo, Q0 + QH_W - qlo

                    def grp_offsets(qh, grp):
                        # pack members tightly; a scores region must not
                        # cross a 512-element PSUM bank boundary
                        pos, offs = 0, []
                        for kt in grp:
                            W = geom(qh, kt)[2]
                            if pos % 512 + min(W, 512) > 512:
                                pos = (pos + 511) // 512 * 512
                            offs.append(pos)
                            pos += W
                        return offs, pos

                    def scores_grp(qh, gi):
                        sl = spool.tile([128, QH_W], F32, tag="sl", name="sl")
                        offs = grp_offsets(qh, GROUPS[qh][gi])[0]
                        for j, kt in enumerate(GROUPS[qh][gi]):
                            K0, qlo, W = geom(qh, kt)
                            base = offs[j]
                            for c0 in range(0, W, 512):
                                cw = min(512, W - c0)
                                nc.tensor.matmul(
                                    sl[:, base + c0 : base + c0 + cw],
                                    KT_sb[prow : prow + 64, mc, K0 : K0 + 128],
                                    QT_sb[
                                        prow : prow + 64, mc,
                                        qlo + c0 : qlo + c0 + cw,
                                    ],
                                    start=True,
                                    stop=True,
                                )
                        return sl

                    hoisted = None
                    for qh in range(2):
                        Q0 = QH_W * qh
                        kmax = 8 if qh == 0 else 16
                        groups = GROUPS[qh]
                        acc = apool.tile([HD + 1, QH_W], F32, tag="acc", name="acc")
                        # software pipeline: scores run one group ahead of PV
                        sl = hoisted if hoisted is not None else scores_grp(qh, 0)
                        hoisted = None
                        for gi, grp in enumerate(groups):
                            goffs, We = grp_offsets(qh, grp)
                            et = epool.tile([128, QH_W], BF16, tag="et", name="et")
                            nc.scalar.activation(
                                et[:, 0:We], sl[:, 0:We], AF.Exp, scale=1.0 / SCALE
                            )
                            if gi + 1 < len(groups):
                                sl = scores_grp(qh, gi + 1)
                            elif qh == 0:
                                # hoist next q-half's first scores ahead of
                                # this group's trailing PV matmuls
                                hoisted = scores_grp(1, 0)
                            for kt in grp:
                                if h == 0 and (qh == 0 or kt >= 8) and kt >= 4:
                                    proj_v_st(kt)  # st == kt; fills vaug for PV
                                for work in sched.get((qh, kt), ()):
                                    work()  # deferred projection tile
                            for j, kt in enumerate(grp):
                                K0, qlo, W = geom(qh, kt)
                                off = qlo - Q0
                                base = goffs[j]
                                if K0 >= Q0:
                                    nc.vector.tensor_mul(
                                        et[:, base : base + 128],
                                        et[:, base : base + 128],
                                        mask_sb[:],
                                    )
                                b0 = off
                                while b0 < QH_W:
                                    b1 = min(QH_W, (b0 // 512 + 1) * 512)
                                    nc.tensor.matmul(
                                        acc[:, b0:b1],
                                        vaug_sb[:, h, kt, :],
                                        et[:, base + b0 - off : base + b1 - off],
                                        start=(kt == 0),
                                        stop=(kt == kmax - 1),
                                        skip_group_check=True,
                                    )
                                    b0 = b1
                        ot = opool.tile([HD + 1, QH_W], F32, tag="ot", name="ot")
                        nc.vector.tensor_copy(ot[:], acc[:])
                        nc.sync.dma_start(
                            out_t[(HD + 1) * h : (HD + 1) * h + HD + 1, Q0 : Q0 + QH_W],
                            ot[:],
                        )

                def qk_tile(mc, sc, s):
                    return lambda: proj_qk_tile(mc, sc, s)

                for _pass in range(PASSES):
                    # prologue: only the tiles the first scores/PV need, V-proj
                    # interleaved to fill DMA-wait bubbles
                    proj_v_st(0)
                    proj_v_st(1)
                    proj_qk_tile(0, 0, KSRC)
                    proj_qk_tile(0, 0, QSRC)
                    proj_qk_tile(0, 1, QSRC)
                    proj_v_st(2)
                    proj_v_st(3)
                    q1 = [qk_tile(1, sc, s) for sc, s in (
                        (0, KSRC), (0, QSRC), (1, QSRC), (1, KSRC),
                        (2, QSRC), (2, KSRC), (3, QSRC), (3, KSRC))]
                    # deferred tiles, placed just before their deadlines in
                    # windows where ACT (exp) is the busier engine
                    attn_head(0, sched={
                        (0, 0): [qk_tile(0, 1, KSRC)],
                        (0, 1): [qk_tile(0, 2, QSRC)],
                        (0, 2): [qk_tile(0, 3, QSRC)],
                        (1, 0): [qk_tile(0, 2, KSRC)],
                        (1, 1): [qk_tile(0, 3, KSRC)],
                        (1, 2): [q1[0]], (1, 3): [q1[1]], (1, 4): [q1[2]],
                        (1, 5): [q1[3]], (1, 6): [q1[4]], (1, 7): [q1[5]],
                    })
                    attn_head(1, sched={(1, 0): [q1[6]], (1, 1): [q1[7]]})
                    attn_head(2)
                    attn_head(3)

    nc.finalize()
    return nc


_NC_CACHE = {}


def _get_nc():
    if "nc" not in _NC_CACHE:
        _NC_CACHE["nc"] = build_nc()
    return _NC_CACHE["nc"]


def make_in_maps(query, key, value, Wq, bq, Wk, bk, Wv, bv):
    query, key, value = (np.asarray(x, np.float32) for x in (query, key, value))
    Wq, Wk, Wv = (np.asarray(x, np.float32) for x in (Wq, Wk, Wv))
    bq, bk, bv = (np.asarray(x, np.float32) for x in (bq, bk, bv))
    mask = np.triu(np.ones((128, 128), np.float32)).astype(BFNP)

    def pack_small(bqs, bks, bvs, m):
        out = np.empty((128, 324), np.float32)
        out[:, 0:2] = bqs.reshape(2, 128).T
        out[:, 2:4] = bks.reshape(2, 128).T
        out[:, 4:260] = np.tile(bvs[None, :], (128, 1))
        out[:, 260:324] = np.ascontiguousarray(m).view(np.float32)
        return out

    in_maps = []
    for c in range(N_CORES):
        b, g = c // 2, c % 2
        sl = slice(DSL * g, DSL * g + DSL)
        in_maps.append(
            {
                "qT": np.ascontiguousarray(query[b].astype(BFNP).T),
                "kTd": np.ascontiguousarray(key[b].astype(BFNP).T),
                "vT": np.ascontiguousarray(value[b].astype(BFNP).T),
                "wqT": np.ascontiguousarray(Wq[sl].astype(BFNP).T),
                "wkT": np.ascontiguousarray(Wk[sl].astype(BFNP).T),
                "wvT": np.ascontiguousarray(Wv[sl].astype(BFNP).T),
                "smallp": pack_small(bq[sl], bk[sl], bv[sl], mask),
            }
        )
    return in_maps


def assemble_output(results):
    out = np.empty((B, S, D), np.float32)
    for c in range(N_CORES):
        b, g = c // 2, c % 2
        ot = results[c]["out_t"]  # [260, 2048]
        for hl in range(HPC):
            blk = ot[(HD + 1) * hl : (HD + 1) * hl + HD]  # [64, S]
            den = ot[(HD + 1) * hl + HD]  # [S]
            h = HPC * g + hl
            out[b, :, HD * h : HD * h + HD] = (blk / den).T
    return out


def run(trace=False, **inputs):
    nc = _get_nc()
    in_maps = make_in_maps(**inputs)
    res = run_bass_kernel_spmd(nc, in_maps, list(range(N_CORES)), trace=trace)
    return assemble_output(res.results), res


def kernel(**inputs) -> np.ndarray:
    out, _ = run(trace=False, **inputs)
    return out
